# revision 13
# baseline (speedup 1.0000x reference)
"""AttentionRetrieval kNN kernel for 8 TRN2 NeuronCores (Bass, raw Block style).

Reference math:
    qp  = query @ Wq.T + bq           (4096, 4096)   [flattened over (D=32, H=128)]
    kp  = support @ Wk.T + bk         (16384, 4096)
    sim = -(|qp|^2 + |kp|^2 - 2 qp@kp.T) / sqrt(128)
    idx, w = top16(sim), softmax(top16 values)

The per-row |qp|^2 shifts all scores of a row equally, so it affects neither
the top-k selection nor the softmax; it is dropped.  Device score:
    score = (2/sqrt(H)) * qp @ kp.T - |kp|^2 / sqrt(H)
identical ordering and weights.

Launch 1 (support sharded 8 x 2048): kpT (transposed kp), qpT (transposed qp,
pre-scaled by 2/sqrt(H)), gamma = -|kp|^2/sqrt(H) -- all on device.
Launch 2 (queries sharded 8 x 512): 512 x 16384 x 4096 fp32 matmul + gamma
add, on-chip top-16 (DVE max8 / find_index8 / match_replace) + softmax.

DMA-completion semaphores are per-ring-slot with at most one outstanding DMA
each (slot reuse is gated on consumer progress via engine semaphores), which
keeps every wait race-free under out-of-order DMA completion.
"""
import sys
sys.path.insert(0, "/opt/trn_rl_repo")
import numpy as np
import concourse.bass as bass
from concourse import mybir
from concourse.bass_utils import run_bass_kernel_spmd

f32 = mybir.dt.float32
u16 = mybir.dt.uint16

N_CORES = 8
NQ, NS, D, H = 4096, 16384, 32, 128
DH = D * H
NQ_SH = NQ // N_CORES           # 512
NS_SH = NS // N_CORES           # 2048
K = 16
SC = 512
SCALE_QP = 2.0 / np.sqrt(H)
SCALE_G = -1.0 / np.sqrt(H)
NEG = -1.0e30
ADD, MUL, SUB = mybir.AluOpType.add, mybir.AluOpType.mult, mybir.AluOpType.subtract


def build_launch1():
    nc = bass.Bass("TRN2", target_bir_lowering=False, debug=False, num_devices=N_CORES)
    supT = nc.dram_tensor("supT", (DH, NS_SH), f32, kind="ExternalInput")
    qT = nc.dram_tensor("qT", (DH, NQ_SH), f32, kind="ExternalInput")
    WkT = nc.dram_tensor("WkT", (H, H), f32, kind="ExternalInput")
    WqT = nc.dram_tensor("WqT", (H, H), f32, kind="ExternalInput")
    bk = nc.dram_tensor("bk", (H, 1), f32, kind="ExternalInput")
    bq = nc.dram_tensor("bq", (H, 1), f32, kind="ExternalInput")
    kpT_out = nc.dram_tensor("kpT", (DH, NS_SH), f32, kind="ExternalOutput")
    qpT_out = nc.dram_tensor("qpT", (DH, NQ_SH), f32, kind="ExternalOutput")
    g_out = nc.dram_tensor("gamma", (1, NS_SH), f32, kind="ExternalOutput")

    supT_v = supT.ap().rearrange("(g p) s -> p g s", p=H)
    qT_v = qT.ap().rearrange("(g p) n -> p g n", p=H)
    kpT_v = kpT_out.ap().rearrange("(g p) s -> p g s", p=H)
    qpT_v = qpT_out.ap().rearrange("(g p) n -> p g n", p=H)

    DG = 4
    NDG = 32 // DG              # 8 input tiles per chunk
    NCH = NS_SH // SC           # 4 s-chunks
    R_IN, R_KP, R_SQ, R_PS, R_G = 3, 4, 4, 4, 2

    sup_sb = [nc.alloc_sbuf_tensor(f"sup{i}", [H, DG, SC], f32) for i in range(R_IN)]
    kp_sb = [nc.alloc_sbuf_tensor(f"kp{i}", [H, SC], f32) for i in range(R_KP)]
    sq_sb = [nc.alloc_sbuf_tensor(f"sq{i}", [H, SC], f32) for i in range(R_SQ)]
    qt_sb = [nc.alloc_sbuf_tensor(f"qt{i}", [H, DG, SC], f32) for i in range(2)]
    WkT_sb = nc.alloc_sbuf_tensor("WkT_sb", [H, H], f32)
    WqT_sb = nc.alloc_sbuf_tensor("WqT_sb", [H, H], f32)
    bk_sb = nc.alloc_sbuf_tensor("bk_sb", [H, 1], f32)
    bq_sb = nc.alloc_sbuf_tensor("bq_sb", [H, 1], f32)
    ones_sb = nc.alloc_sbuf_tensor("ones_sb", [H, 1], f32)
    g_sb = [nc.alloc_sbuf_tensor(f"g{i}", [1, SC], f32) for i in range(R_G)]

    ps_k = [nc.alloc_psum_tensor(f"psk{i}", [H, SC], f32) for i in range(R_PS)]
    ps_k2 = nc.alloc_psum_tensor("ps_ksq", [1, SC], f32)
    ps_q = [nc.alloc_psum_tensor(f"psq{i}", [H, SC], f32) for i in range(2)]

    with (
        nc.Block() as block,
        nc.semaphore("s_const") as s_const,
        nc.semaphore("s_sup0") as s_sup0,
        nc.semaphore("s_sup1") as s_sup1,
        nc.semaphore("s_sup2") as s_sup2,
        nc.semaphore("s_qt0") as s_qt0,
        nc.semaphore("s_qt1") as s_qt1,
        nc.semaphore("s_kpo0") as s_kpo0,
        nc.semaphore("s_kpo1") as s_kpo1,
        nc.semaphore("s_kpo2") as s_kpo2,
        nc.semaphore("s_kpo3") as s_kpo3,
        nc.semaphore("s_gout") as s_gout,
        nc.semaphore("pe") as pe,
        nc.semaphore("pe2") as pe2,
        nc.semaphore("dve") as dve,
        nc.semaphore("act") as act,
        nc.semaphore("gam") as gam,
    ):
        s_sup = [s_sup0, s_sup1, s_sup2]
        s_qt = [s_qt0, s_qt1]
        s_kpo = [s_kpo0, s_kpo1, s_kpo2, s_kpo3]

        @block.sync
        def _(sync):
            for src, sb in ((WkT, WkT_sb), (WqT, WqT_sb), (bk, bk_sb), (bq, bq_sb)):
                sync.dma_start(out=sb[:], in_=src.ap()).then_inc(s_const, 16)

            def kp_out(g):
                c, d = divmod(g, 32)
                sync.wait_ge(dve, g + 1)
                sync.dma_start(
                    out=kpT_v[:, d, c * SC:(c + 1) * SC], in_=kp_sb[g % R_KP][:]
                ).then_inc(s_kpo[g % R_KP], 16)

            for c in range(NCH):
                if c >= 1:
                    # deferred gamma out of the previous chunk
                    sync.wait_ge(gam, c)
                    sync.dma_start(
                        out=g_out.ap()[:, (c - 1) * SC:c * SC],
                        in_=g_sb[(c - 1) % R_G][:],
                    ).then_inc(s_gout, 16)
                for i in range(NDG):
                    t = c * NDG + i
                    if t >= R_IN:
                        sync.wait_ge(pe, DG * (t - R_IN + 1))
                    sync.dma_start(
                        out=sup_sb[t % R_IN][:],
                        in_=supT_v[:, i * DG:(i + 1) * DG, c * SC:(c + 1) * SC],
                    ).then_inc(s_sup[t % R_IN], 16)
                    # kp-outs of the previous input tile's d's
                    if i >= 1:
                        for d in range((i - 1) * DG, i * DG):
                            kp_out(c * 32 + d)
                for d in range((NDG - 1) * DG, 32):
                    kp_out(c * 32 + d)
            for i in range(NDG):
                if i >= 2:
                    sync.wait_ge(pe, 128 + DG * (i - 1))
                sync.dma_start(
                    out=qt_sb[i % 2][:], in_=qT_v[:, i * DG:(i + 1) * DG, :]
                ).then_inc(s_qt[i % 2], 16)
                for j in range(DG):
                    d = (i - 1) * DG + j
                    if i >= 1:
                        gq = 128 + d
                        sync.wait_ge(dve, gq + 1)
                        sync.dma_start(
                            out=qpT_v[:, d, :], in_=kp_sb[gq % R_KP][:]
                        ).then_inc(s_kpo[gq % R_KP], 16)
            for d in range((NDG - 1) * DG, 32):
                gq = 128 + d
                sync.wait_ge(dve, gq + 1)
                sync.dma_start(
                    out=qpT_v[:, d, :], in_=kp_sb[gq % R_KP][:]
                ).then_inc(s_kpo[gq % R_KP], 16)
            sync.wait_ge(gam, NCH)
            sync.dma_start(
                out=g_out.ap()[:, (NCH - 1) * SC:NCH * SC],
                in_=g_sb[(NCH - 1) % R_G][:],
            ).then_inc(s_gout, 16)
            # final: wait for all outstanding output DMAs
            for sl in range(R_KP):
                n_out = len([g for g in range(160) if g % R_KP == sl])
                sync.wait_ge(s_kpo[sl], 16 * n_out)
            sync.wait_ge(s_gout, 16 * NCH)

        @block.tensor
        def _(tensor):
            def mm_ones(c, d):
                g = c * 32 + d
                if d == 0:
                    tensor.wait_ge(gam, c)
                tensor.wait_ge(act, g + 1)
                nc.tensor.matmul(
                    ps_k2[:], lhsT=ones_sb[:], rhs=sq_sb[g % R_SQ][:],
                    start=(d == 0), stop=(d == 31),
                ).then_inc(pe2, 1)

            tensor.wait_ge(s_const, 4 * 16)
            for c in range(NCH):
                for d in range(32):
                    i, j = d // DG, d % DG
                    t = c * NDG + i
                    g = c * 32 + d
                    if j == 0:
                        tensor.wait_ge(s_sup[t % R_IN], 16 * (t // R_IN + 1))
                    if g >= R_PS:
                        tensor.wait_ge(dve, g - R_PS + 1)
                    nc.tensor.matmul(
                        ps_k[g % R_PS][:], lhsT=WkT_sb[:],
                        rhs=sup_sb[t % R_IN][:, j, :],
                        start=True, stop=True,
                    ).then_inc(pe, 1)
                    if d >= 2:
                        mm_ones(c, d - 2)
                mm_ones(c, 30)
                mm_ones(c, 31)
            for d in range(32):
                i, j = d // DG, d % DG
                if j == 0:
                    tensor.wait_ge(s_qt[i % 2], 16 * (i // 2 + 1))
                if d >= 2:
                    tensor.wait_ge(dve, 128 + (d - 2) + 1)
                nc.tensor.matmul(
                    ps_q[d % 2][:], lhsT=WqT_sb[:], rhs=qt_sb[i % 2][:, j, :],
                    start=True, stop=True,
                ).then_inc(pe, 1)

        @block.vector
        def _(vector):
            vector.wait_ge(s_const, 4 * 16)
            nc.vector.memset(ones_sb[:], 1.0)
            for c in range(NCH):
                for d in range(32):
                    g = c * 32 + d
                    vector.wait_ge(pe, g + 1)
                    if g >= R_KP:
                        gp = g - R_KP
                        vector.wait_ge(s_kpo[g % R_KP], 16 * (gp // R_KP + 1))
                        vector.wait_ge(act, gp + 1)
                    nc.vector.tensor_scalar(
                        kp_sb[g % R_KP][:], ps_k[g % R_PS][:], bk_sb[:], None, ADD
                    ).then_inc(dve, 1)
                vector.wait_ge(pe2, 32 * (c + 1))
                if c >= R_G:
                    vector.wait_ge(s_gout, 16 * (c - R_G + 1))
                nc.vector.tensor_scalar(
                    g_sb[c % R_G][:], ps_k2[:], float(SCALE_G), None, MUL
                ).then_inc(gam, 1)
            for d in range(32):
                gq = 128 + d
                gp = gq - R_KP
                vector.wait_ge(pe, gq + 1)
                vector.wait_ge(s_kpo[gq % R_KP], 16 * (gp // R_KP + 1))
                if gp < 128:
                    vector.wait_ge(act, gp + 1)
                nc.vector.tensor_scalar(
                    kp_sb[gq % R_KP][:], ps_q[d % 2][:], bq_sb[:], float(SCALE_QP),
                    ADD, op1=MUL,
                ).then_inc(dve, 1)

        @block.scalar
        def _(scalar):
            for c in range(NCH):
                for d in range(32):
                    g = c * 32 + d
                    scalar.wait_ge(dve, g + 1)
                    if g >= R_SQ:
                        scalar.wait_ge(pe2, g - R_SQ + 1)
                    nc.scalar.activation(
                        sq_sb[g % R_SQ][:], kp_sb[g % R_KP][:],
                        mybir.ActivationFunctionType.Square,
                    ).then_inc(act, 1)

    return nc


def build_launch2():
    nc = bass.Bass("TRN2", target_bir_lowering=False, debug=False, num_devices=N_CORES)
    kpT = nc.dram_tensor("kpT", (DH, NS), f32, kind="ExternalInput")
    qpT = nc.dram_tensor("qpT", (DH, NQ_SH), f32, kind="ExternalInput")
    gbc = nc.dram_tensor("gbc", (H, NS), f32, kind="ExternalInput")
    idx_out = nc.dram_tensor("idx", (4, H, K), u16, kind="ExternalOutput")
    w_out = nc.dram_tensor("w", (4, H, K), f32, kind="ExternalOutput")

    kpT_v = kpT.ap().rearrange("(g p) s -> p g s", p=H)
    qpT_v = qpT.ap().rearrange("(g p) n -> p g n", p=H)

    NSC = NS // SC              # 32 chunks per half
    DG = 4
    NDG = 32 // DG              # 8 kt tiles per chunk
    R_KT, R_G, R_PS = 4, 4, 4
    NT = 2 * NSC * NDG          # 512 kt tiles total

    kt_sb = [nc.alloc_sbuf_tensor(f"kt{i}", [H, DG, SC], f32) for i in range(R_KT)]
    g_sb = [nc.alloc_sbuf_tensor(f"gs{i}", [H, SC], f32) for i in range(R_G)]
    qp_sb = nc.alloc_sbuf_tensor("qp_sb", [H, 32, 256], f32)
    score = [nc.alloc_sbuf_tensor(f"score{b}", [H, NS], f32) for b in range(2)]
    mm_sb = [nc.alloc_sbuf_tensor(f"mm{g}", [H, K], f32) for g in range(4)]
    ii_sb = [nc.alloc_sbuf_tensor(f"ii{g}", [H, K], u16) for g in range(4)]
    sm_sb = [nc.alloc_sbuf_tensor(f"sm{g}", [H, K], f32) for g in range(4)]
    ex_sb = [nc.alloc_sbuf_tensor(f"ex{g}", [H, K], f32) for g in range(4)]
    w_sb = [nc.alloc_sbuf_tensor(f"wv{g}", [H, K], f32) for g in range(4)]
    sum_sb = [nc.alloc_sbuf_tensor(f"su{g}", [H, 1], f32) for g in range(4)]
    rs_sb = [nc.alloc_sbuf_tensor(f"rrs{g}", [H, 1], f32) for g in range(4)]

    ps = [nc.alloc_psum_tensor(f"ps{i}", [H, SC], f32) for i in range(R_PS)]

    with (
        nc.Block() as block,
        nc.semaphore("s_qp") as s_qp,
        nc.semaphore("s_kt0") as s_kt0,
        nc.semaphore("s_kt1") as s_kt1,
        nc.semaphore("s_kt2") as s_kt2,
        nc.semaphore("s_kt3") as s_kt3,
        nc.semaphore("s_g0") as s_g0,
        nc.semaphore("s_g1") as s_g1,
        nc.semaphore("s_g2") as s_g2,
        nc.semaphore("s_g3") as s_g3,
        nc.semaphore("s_out") as s_out,
        nc.semaphore("pe") as pe,
        nc.semaphore("pet") as pet,
        nc.semaphore("dve") as dve,
        nc.semaphore("tk") as tk,
        nc.semaphore("act") as act,
        nc.semaphore("rdy") as rdy,
    ):
        s_kt = [s_kt0, s_kt1, s_kt2, s_kt3]
        s_g = [s_g0, s_g1, s_g2, s_g3]

        @block.sync
        def _(sync):
            n_out = 0
            for half in range(2):
                if half == 1:
                    sync.wait_ge(pet, NDG * NSC)   # all half-0 tiles consumed
                sync.dma_start(
                    out=qp_sb[:], in_=qpT_v[:, :, half * 256:(half + 1) * 256]
                ).then_inc(s_qp, 16)
                for sc in range(NSC):
                    S = half * NSC + sc
                    for i in range(NDG):
                        t = S * NDG + i
                        if t >= R_KT:
                            sync.wait_ge(pet, t - R_KT + 1)
                        sync.dma_start(
                            out=kt_sb[t % R_KT][:],
                            in_=kpT_v[:, i * DG:(i + 1) * DG, sc * SC:(sc + 1) * SC],
                        ).then_inc(s_kt[t % R_KT], 16)
                    if S >= R_G:
                        sync.wait_ge(dve, 2 * (S - R_G + 1))
                    sync.dma_start(
                        out=g_sb[S % R_G][:], in_=gbc.ap()[:, sc * SC:(sc + 1) * SC]
                    ).then_inc(s_g[S % R_G], 16)
            for gblk in range(4):
                sync.wait_ge(rdy, gblk + 1)
                sync.dma_start(out=idx_out.ap()[gblk], in_=ii_sb[gblk][:]).then_inc(s_out, 16)
                sync.dma_start(out=w_out.ap()[gblk], in_=w_sb[gblk][:]).then_inc(s_out, 16)
                n_out += 2
            sync.wait_ge(s_out, 16 * n_out)

        @block.tensor
        def _(tensor):
            for half in range(2):
                tensor.wait_ge(s_qp, 16 * (half + 1))
                for sc in range(NSC):
                    S = half * NSC + sc
                    for d in range(32):
                        i, j = d // DG, d % DG
                        t = S * NDG + i
                        if j == 0:
                            tensor.wait_ge(s_kt[t % R_KT], 16 * (t // R_KT + 1))
                        for b in range(2):
                            g = 2 * S + b
                            if d == 0 and g >= R_PS:
                                tensor.wait_ge(dve, g - R_PS + 1)
                            inst = nc.tensor.matmul(
                                ps[g % R_PS][:],
                                lhsT=qp_sb[:, d, b * H:(b + 1) * H],
                                rhs=kt_sb[t % R_KT][:, j, :],
                                start=(d == 0), stop=(d == 31),
                            )
                            if d == 31 and b == 0:
                                inst.then_inc(pe, 1)   # pe = #completed b0 groups
                            if j == DG - 1 and b == 1:
                                inst.then_inc(pet, 1)  # pet = #fully-consumed kt tiles

        @block.vector
        def _(vector):
            for half in range(2):
                for sc in range(NSC):
                    S = half * NSC + sc
                    if half == 1 and sc == 0:
                        # WAR: half-0 top-k reads of score must finish first
                        vector.wait_ge(tk, 14)
                    for b in range(2):
                        g = 2 * S + b
                        if b == 0:
                            vector.wait_ge(pe, S + 1)
                            vector.wait_ge(s_g[S % R_G], 16 * (S // R_G + 1))
                        else:
                            vector.wait_ge(pet, NDG * (S + 1))
                        nc.vector.tensor_tensor(
                            out=score[b][:, sc * SC:(sc + 1) * SC],
                            in0=ps[g % R_PS][:], in1=g_sb[S % R_G][:], op=ADD,
                        ).then_inc(dve, 1)
                # all score-chunk adds of this half must be fully complete
                vector.wait_ge(dve, 2 * NSC * (half + 1))
                for b in range(2):
                    gblk = half * 2 + b
                    base = 7 * gblk
                    s = score[b]
                    nc.vector.max(out=mm_sb[gblk][:, 0:8], in_=s[:]).then_inc(tk, 1)
                    vector.wait_ge(tk, base + 1)
                    nc.vector.max_index(
                        out=ii_sb[gblk][:, 0:8], in_max=mm_sb[gblk][:, 0:8], in_values=s[:]
                    ).then_inc(tk, 1)
                    vector.wait_ge(tk, base + 2)
                    nc.vector.match_replace(
                        out=s[:], in_to_replace=mm_sb[gblk][:, 0:8], in_values=s[:],
                        imm_value=NEG,
                    ).then_inc(tk, 1)
                    vector.wait_ge(tk, base + 3)
                    nc.vector.max(out=mm_sb[gblk][:, 8:16], in_=s[:]).then_inc(tk, 1)
                    vector.wait_ge(tk, base + 4)
                    nc.vector.max_index(
                        out=ii_sb[gblk][:, 8:16], in_max=mm_sb[gblk][:, 8:16], in_values=s[:]
                    ).then_inc(tk, 1)
                    vector.wait_ge(tk, base + 5)
                    nc.vector.tensor_scalar(
                        sm_sb[gblk][:], mm_sb[gblk][:], mm_sb[gblk][:, 0:1], None, SUB
                    ).then_inc(tk, 1)
                    vector.wait_ge(act, gblk + 1)
                    nc.vector.reciprocal(rs_sb[gblk][:], sum_sb[gblk][:]).then_inc(tk, 1)
                    vector.wait_ge(tk, base + 7)
                    nc.vector.tensor_scalar(
                        w_sb[gblk][:], ex_sb[gblk][:], rs_sb[gblk][:], None, MUL
                    ).then_inc(rdy, 1)

        @block.scalar
        def _(scalar):
            for gblk in range(4):
                scalar.wait_ge(tk, 7 * gblk + 6)
                nc.scalar.activation(
                    ex_sb[gblk][:], sm_sb[gblk][:],
                    mybir.ActivationFunctionType.Exp,
                    accum_out=sum_sb[gblk][:],
                ).then_inc(act, 1)

    return nc


_CACHE = {}


def _get_programs():
    if "l1" not in _CACHE:
        _CACHE["l1"] = build_launch1()
        _CACHE["l2"] = build_launch2()
    return _CACHE["l1"], _CACHE["l2"]


def run_launches(query, support, Wq, bq, Wk, bk, trace2=False):
    nc1, nc2 = _get_programs()

    supT = np.ascontiguousarray(support.reshape(NS, DH).T)
    qT = np.ascontiguousarray(query.reshape(NQ, DH).T)
    WkT_a = np.ascontiguousarray(Wk.T)
    WqT_a = np.ascontiguousarray(Wq.T)
    bk_c = np.ascontiguousarray(bk.reshape(H, 1))
    bq_c = np.ascontiguousarray(bq.reshape(H, 1))

    in_maps1 = [
        {
            "supT": np.ascontiguousarray(supT[:, c * NS_SH:(c + 1) * NS_SH]),
            "qT": np.ascontiguousarray(qT[:, c * NQ_SH:(c + 1) * NQ_SH]),
            "WkT": WkT_a, "WqT": WqT_a, "bk": bk_c, "bq": bq_c,
        }
        for c in range(N_CORES)
    ]
    res1 = run_bass_kernel_spmd(nc1, in_maps1, core_ids=list(range(N_CORES)))

    kpT_full = np.concatenate([res1.results[c]["kpT"] for c in range(N_CORES)], axis=1)
    gamma = np.concatenate([res1.results[c]["gamma"][0] for c in range(N_CORES)])
    gbc_a = np.ascontiguousarray(np.broadcast_to(gamma, (H, NS)))

    in_maps2 = [
        {"kpT": kpT_full, "gbc": gbc_a, "qpT": res1.results[c]["qpT"]}
        for c in range(N_CORES)
    ]
    res2 = run_bass_kernel_spmd(
        nc2, in_maps2, core_ids=list(range(N_CORES)), trace=trace2
    )

    idx = np.concatenate(
        [res2.results[c]["idx"].reshape(NQ_SH, K) for c in range(N_CORES)], axis=0
    ).astype(np.int32)
    w = np.concatenate(
        [res2.results[c]["w"].reshape(NQ_SH, K) for c in range(N_CORES)], axis=0
    )
    return idx, w, (res1, res2)


def kernel(query, support, Wq, bq, Wk, bk, k):
    assert int(k) == K
    query = np.asarray(query, np.float32)
    support = np.asarray(support, np.float32)
    Wq = np.asarray(Wq, np.float32)
    bq = np.asarray(bq, np.float32)
    Wk = np.asarray(Wk, np.float32)
    bk = np.asarray(bk, np.float32)
    idx, w, _ = run_launches(query, support, Wq, bq, Wk, bk)
    return idx, w
